# revision 1
# baseline (speedup 1.0000x reference)
"""Linear attention Bass kernel for Trainium2 (8 NeuronCores).

Problem: x [4, 8192, 1024] f32, W [1024, 3072] f32.
  qkv = x @ W; q,k,v = split(qkv); q,k = elu(.)+1
  KV = einsum('bld,blh->bhd', k, v); ksum = k.sum(1)
  Z = 1/(q.ksum + eps); V = einsum('bld,bhd,bl->blh', q, KV, Z)

Sharding: 8 cores, core c handles batch b=c//2, sequence half h=c%2
(4096 rows each).  KV / ksum reductions span the full batch sequence, so
the two cores of a pair AllReduce their partial KV^T [1024,1024] + ksum
(4.2 MB fp32) in-NEFF.  Fallback (USE_CC=False): each core redundantly
computes k,v for the sibling half (no collectives).

Per-core dataflow (all matmuls bf16 inputs, fp32 PSUM accumulation):
  phase 1: stream xT tiles; q^T = Wq^T-form matmul (comes out [d,l] ready
           for phase 3), k,v = standard form [l,d]; phi=elu+1 via
           exp/min/max; q^T -> DRAM stash, k,v -> DRAM stash;
           ksum accumulated in PSUM via ones-vector matmul.
  phase 2: KV^T[d,h] += k_tile^T-free matmul over all l chunks, h in two
           512 halves (PSUM = 8 banks per half); partial KV^T + ksum ->
           cc buffer; AllReduce over core pairs.
  phase 3: V[l,:] = (q^T)^T @ KV^T, denominator from ksum column matmul,
           z = 1/(den+eps), scale, DMA out.
"""

import numpy as np
import ml_dtypes

import concourse.bass as bass
import concourse.tile as tile
from concourse import mybir
from concourse.bacc import Bacc

USE_CC = True
TRACE = False
LAST_RESULTS = None

B, L, D = 4, 8192, 1024
NCORES = 8
R = 4096              # rows per core
LT = 512              # l-tile width (columns of xT per tile)
EPS = 1e-6

BF16 = mybir.dt.bfloat16
F32 = mybir.dt.float32
NPBF16 = ml_dtypes.bfloat16

_NC_CACHE = {}


def _emit_phi(nc, pool_e, out_bf, psum_in, width):
    """out_bf (bf16) = elu(psum_in)+1 = min(exp(y),1) + max(y,0).

    Ops are emitted per 512-wide slice so each reads a single PSUM bank
    (one stop-matmul dep); the combine reads only SBUF tiles.  Keeps the
    per-instruction semaphore-wait count under the ISA limit.
    """
    for s in range(0, width, 512):
        w = min(512, width - s)
        ps = psum_in[:, s : s + w]
        e = pool_e.tile([128, w], F32, tag=f"phi_e_{w}_{s}", name=f"e{w}_{s}")
        nc.scalar.activation(out=e, in_=ps, func=mybir.ActivationFunctionType.Exp)
        r = pool_e.tile([128, w], F32, tag=f"phi_r_{w}_{s}", name=f"r{w}_{s}")
        nc.vector.tensor_scalar(
            out=r, in0=ps, scalar1=0.0, scalar2=None, op0=mybir.AluOpType.max
        )
        nc.vector.scalar_tensor_tensor(
            out=out_bf[:, s : s + w],
            in0=e,
            scalar=1.0,
            in1=r,
            op0=mybir.AluOpType.min,
            op1=mybir.AluOpType.add,
        )


def build_bass(use_cc=USE_CC):
    nc = Bacc(trn_type="TRN2", num_devices=NCORES)

    n_xt_cols = R if use_cc else 2 * R
    n_lc = n_xt_cols // 128          # 32 or 64 chunks of 128 rows
    n_tiles = n_xt_cols // LT        # 8 or 16 l-tiles
    local_tiles = R // LT            # 8 tiles that produce q/output

    xt = nc.dram_tensor("xt", [128, 8, n_xt_cols], BF16, kind="ExternalInput")
    wq = nc.dram_tensor("wq", [128, 8, 1024], BF16, kind="ExternalInput")
    wkv = nc.dram_tensor("wkv", [128, 8, 2048], BF16, kind="ExternalInput")
    out = nc.dram_tensor("out", [R, 1024], F32, kind="ExternalOutput")

    q_dram = nc.dram_tensor("q_stash", [128, 8, R], BF16)
    k_dram = nc.dram_tensor("k_stash", [n_lc, 128, 1024], BF16)
    v_dram = nc.dram_tensor("v_stash", [n_lc, 128, 1024], BF16)
    if use_cc:
        # row 128 of each [129, 1024] chunk holds ksum[m*128:(m+1)*128] in
        # cols 0:128 (rest zeros, harmlessly allreduced).
        cc_in = nc.dram_tensor("cc_in", [8, 129, 1024], F32)
        cc_out = nc.dram_tensor("cc_out", [8, 129, 1024], F32)
        ks_src = cc_out
    else:
        ks_dram = nc.dram_tensor("ks_stash", [8, 128], F32)

    mm = nc.tensor.matmul
    Act = mybir.ActivationFunctionType

    with tile.TileContext(nc) as tc:
        with tc.tile_pool(name="consts", bufs=1) as consts:
            wq_sb = consts.tile([128, 8, 1024], BF16)
            nc.sync.dma_start(out=wq_sb, in_=wq[:])
            wkv_sb = consts.tile([128, 8, 2048], BF16)
            nc.sync.dma_start(out=wkv_sb, in_=wkv[:])
            ones_sb = consts.tile([128, 1], BF16)
            nc.vector.memset(ones_sb, 1.0)

            # ---------------- phase 1: qkv + phi + stashes + ksum ---------
            with (
                tc.tile_pool(name="xt_p", bufs=3) as xt_p,
                tc.tile_pool(name="qout_p", bufs=2) as qout_p,
                tc.tile_pool(name="e_p", bufs=4) as e_p,
                tc.tile_pool(name="kt_p", bufs=3) as kt_p,
                tc.tile_pool(name="vt_p", bufs=3) as vt_p,
                tc.tile_pool(name="q_ps_p", bufs=2, space="PSUM") as q_ps_p,
                tc.tile_pool(name="kv_ps_p", bufs=1, space="PSUM") as kv_ps_p,
                tc.tile_pool(name="ks_ps_p", bufs=1, space="PSUM") as ks_ps_p,
            ):
                ksum_ps = [
                    ks_ps_p.tile([1, 512], F32, tag=f"ks{h}", name=f"ks{h}")
                    for h in range(2)
                ]

                def q_block(xt_tile, qout, m):
                    pq = q_ps_p.tile([128, LT], F32)
                    for k in range(8):
                        mm(
                            pq,
                            lhsT=wq_sb[:, k, m * 128 : (m + 1) * 128],
                            rhs=xt_tile[:, k, :],
                            start=(k == 0),
                            stop=(k == 7),
                        )
                    _emit_phi(nc, e_p, qout[:, m, :], pq, LT)

                def kv_block(xt_tile, t, lc):
                    idx = t * 4 + lc
                    # four independent single-bank PSUM tiles: each reader
                    # then carries exactly one stop-matmul dependency.
                    pkv = [
                        kv_ps_p.tile([128, 512], F32, tag=f"pkv{n}", name=f"pkv{n}")
                        for n in range(4)
                    ]
                    for k in range(8):
                        lhsT = xt_tile[:, k, lc * 128 : (lc + 1) * 128]
                        for n in range(4):
                            mm(
                                pkv[n],
                                lhsT=lhsT,
                                rhs=wkv_sb[:, k, n * 512 : (n + 1) * 512],
                                start=(k == 0),
                                stop=(k == 7),
                            )
                    kt = kt_p.tile([128, 1024], BF16)
                    for s in range(2):
                        _emit_phi(nc, e_p, kt[:, s * 512 : (s + 1) * 512], pkv[s], 512)
                    vt = vt_p.tile([128, 1024], BF16)
                    for s in range(2):
                        nc.scalar.activation(
                            out=vt[:, s * 512 : (s + 1) * 512],
                            in_=pkv[2 + s],
                            func=Act.Copy,
                        )
                    nc.sync.dma_start(out=k_dram[idx], in_=kt)
                    nc.sync.dma_start(out=v_dram[idx], in_=vt)
                    for h in range(2):
                        mm(
                            ksum_ps[h],
                            lhsT=ones_sb,
                            rhs=kt[:, h * 512 : (h + 1) * 512],
                            start=(idx == 0),
                            stop=(idx == n_lc - 1),
                        )

                for t in range(n_tiles):
                    xt_tile = xt_p.tile([128, 8, LT], BF16)
                    nc.sync.dma_start(
                        out=xt_tile, in_=xt[:, :, t * LT : (t + 1) * LT]
                    )
                    if t < local_tiles:
                        qout = qout_p.tile([128, 8, LT], BF16)
                        for seg in range(4):
                            q_block(xt_tile, qout, 2 * seg)
                            q_block(xt_tile, qout, 2 * seg + 1)
                            kv_block(xt_tile, t, seg)
                        nc.sync.dma_start(
                            out=q_dram[:, :, t * LT : (t + 1) * LT], in_=qout
                        )
                    else:
                        for lc in range(4):
                            kv_block(xt_tile, t, lc)

                # stash ksum (psum) to DRAM before phase-1 psum pools close
                ks_sb = consts.tile([1, 1024], F32)
                for h in range(2):
                    nc.vector.tensor_copy(
                        out=ks_sb[:, h * 512 : (h + 1) * 512], in_=ksum_ps[h]
                    )
                for m in range(8):
                    src = ks_sb[0:1, m * 128 : (m + 1) * 128]
                    if use_cc:
                        nc.sync.dma_start(out=cc_in[m, 128, 0:128], in_=src)
                    else:
                        nc.sync.dma_start(out=ks_dram[m, :], in_=src)

            # ---------------- phase 2: KV^T accumulation ------------------
            with tc.tile_pool(name="p23", bufs=1) as p23:
                if not use_cc:
                    kvt_bf = p23.tile([128, 8, 1024], BF16)
                with (
                    tc.tile_pool(name="k2_p", bufs=6) as k2_p,
                    tc.tile_pool(name="v2_p", bufs=6) as v2_p,
                    tc.tile_pool(name="kvt_ps_p", bufs=1, space="PSUM") as kvt_ps_p,
                ):
                    for half in range(2):
                        kvt_ps = [
                            kvt_ps_p.tile(
                                [128, 512], F32, tag=f"kvt{m}", name=f"kvt{m}"
                            )
                            for m in range(8)
                        ]
                        for lc in range(n_lc):
                            kt2 = k2_p.tile([128, 1024], BF16)
                            nc.sync.dma_start(out=kt2, in_=k_dram[lc])
                            vt2 = v2_p.tile([128, 512], BF16)
                            nc.sync.dma_start(
                                out=vt2,
                                in_=v_dram[lc][:, half * 512 : (half + 1) * 512],
                            )
                            for m in range(8):
                                mm(
                                    kvt_ps[m],
                                    lhsT=kt2[:, m * 128 : (m + 1) * 128],
                                    rhs=vt2,
                                    start=(lc == 0),
                                    stop=(lc == n_lc - 1),
                                )
                        for m in range(8):
                            if use_cc:
                                kvs = k2_p.tile(
                                    [128, 512], F32, tag="kvs", name=f"kvs{half}_{m}"
                                )
                                nc.scalar.activation(
                                    out=kvs, in_=kvt_ps[m], func=Act.Copy
                                )
                                nc.sync.dma_start(
                                    out=cc_in[
                                        m, 0:128, half * 512 : (half + 1) * 512
                                    ],
                                    in_=kvs,
                                )
                            else:
                                nc.vector.tensor_copy(
                                    out=kvt_bf[:, m, half * 512 : (half + 1) * 512],
                                    in_=kvt_ps[m],
                                )

                if use_cc:
                    nc.gpsimd.collective_compute(
                        "AllReduce",
                        mybir.AluOpType.add,
                        replica_groups=[[0, 1], [2, 3], [4, 5], [6, 7]],
                        ins=[cc_in[:]],
                        outs=[cc_out[:]],
                    )

                # ---------------- phase 3: output -------------------------
                with (
                    tc.tile_pool(name="p3", bufs=1) as p3,
                    tc.tile_pool(name="qt_p", bufs=2) as qt_p,
                    tc.tile_pool(name="ob_p", bufs=3) as ob_p,
                    tc.tile_pool(name="z_p", bufs=4) as z_p,
                    tc.tile_pool(name="pv_ps_p", bufs=2, space="PSUM") as pv_ps_p,
                    tc.tile_pool(name="pd_ps_p", bufs=2, space="PSUM") as pd_ps_p,
                ):
                    if use_cc:
                        kvt_f = p3.tile([128, 8, 1024], F32)
                        for m in range(8):
                            nc.sync.dma_start(
                                out=kvt_f[:, m, :], in_=cc_out[m, 0:128, :]
                            )
                        kvt_bf = p3.tile([128, 8, 1024], BF16)
                        for m in range(8):
                            nc.vector.tensor_copy(
                                out=kvt_bf[:, m, :], in_=kvt_f[:, m, :]
                            )
                    ksum_f = p3.tile([128, 8], F32)
                    for m in range(8):
                        if use_cc:
                            nc.sync.dma_start(
                                out=ksum_f[:, m : m + 1], in_=cc_out[m, 128, 0:128]
                            )
                        else:
                            nc.sync.dma_start(
                                out=ksum_f[:, m : m + 1], in_=ks_dram[m, :]
                            )
                    ksum_b = p3.tile([128, 8], BF16)
                    for m in range(8):
                        nc.vector.tensor_copy(
                            out=ksum_b[:, m : m + 1], in_=ksum_f[:, m : m + 1]
                        )

                    for g in range(8):
                        qt = qt_p.tile([128, 8, 512], BF16)
                        nc.sync.dma_start(
                            out=qt, in_=q_dram[:, :, g * 512 : (g + 1) * 512]
                        )
                        for lc in range(4):
                            pv0 = pv_ps_p.tile([128, 512], F32, tag="pv0")
                            pv1 = pv_ps_p.tile([128, 512], F32, tag="pv1")
                            pd = pd_ps_p.tile([128, 1], F32)
                            for k in range(8):
                                lhsT = qt[:, k, lc * 128 : (lc + 1) * 128]
                                st, sp = (k == 0), (k == 7)
                                mm(pv0, lhsT=lhsT, rhs=kvt_bf[:, k, 0:512],
                                   start=st, stop=sp)
                                mm(pv1, lhsT=lhsT, rhs=kvt_bf[:, k, 512:1024],
                                   start=st, stop=sp)
                                mm(pd, lhsT=lhsT, rhs=ksum_b[:, k : k + 1],
                                   start=st, stop=sp)
                            z = z_p.tile([128, 1], F32)
                            nc.vector.tensor_scalar(
                                out=z, in0=pd, scalar1=EPS, scalar2=None,
                                op0=mybir.AluOpType.add,
                            )
                            nc.vector.reciprocal(out=z, in_=z)
                            ob = ob_p.tile([128, 1024], F32)
                            nc.vector.tensor_scalar_mul(
                                out=ob[:, 0:512], in0=pv0, scalar1=z
                            )
                            nc.vector.tensor_scalar_mul(
                                out=ob[:, 512:1024], in0=pv1, scalar1=z
                            )
                            r0 = (g * 4 + lc) * 128
                            nc.sync.dma_start(out=out[r0 : r0 + 128, :], in_=ob)
    if not nc.is_finalized():
        nc.finalize()
    return nc


def _get_nc(use_cc):
    if use_cc not in _NC_CACHE:
        _NC_CACHE[use_cc] = build_bass(use_cc)
    return _NC_CACHE[use_cc]


def _prep_inputs(x, W, use_cc):
    """Build per-core input maps (host-side shard + transpose + bf16 cast)."""
    wq_h = np.ascontiguousarray(
        W[:, :1024].reshape(8, 128, 1024).transpose(1, 0, 2)
    ).astype(NPBF16)
    wkv_h = np.ascontiguousarray(
        W[:, 1024:].reshape(8, 128, 2048).transpose(1, 0, 2)
    ).astype(NPBF16)

    in_maps = []
    for c in range(NCORES):
        b, half = divmod(c, 2)
        xb = x[b]  # [8192, 1024]
        if use_cc:
            rows = xb[half * R : (half + 1) * R]           # [4096, 1024]
        else:
            own = xb[half * R : (half + 1) * R]
            sib = xb[(1 - half) * R : (2 - half) * R]
            rows = np.concatenate([own, sib], axis=0)       # [8192, 1024]
        # -> xT [1024, n] -> [8, 128, n] -> [128, 8, n]
        xt_h = np.ascontiguousarray(
            rows.T.reshape(8, 128, rows.shape[0]).transpose(1, 0, 2)
        ).astype(NPBF16)
        in_maps.append({"xt": xt_h, "wq": wq_h, "wkv": wkv_h})
    return in_maps


def kernel(x, W):
    global LAST_RESULTS
    from concourse.bass_utils import run_bass_kernel_spmd

    x = np.asarray(x, dtype=np.float32)
    W = np.asarray(W, dtype=np.float32)
    nc = _get_nc(USE_CC)
    in_maps = _prep_inputs(x, W, USE_CC)
    try:
        res = run_bass_kernel_spmd(
            nc, in_maps, core_ids=list(range(NCORES)), trace=TRACE
        )
    except ModuleNotFoundError:
        # NTFF profiling hook unavailable (axon client without antenv.axon_hooks)
        res = run_bass_kernel_spmd(
            nc, in_maps, core_ids=list(range(NCORES)), trace=False
        )
    LAST_RESULTS = res
    out = np.empty((B, L, D), dtype=np.float32)
    for c in range(NCORES):
        b, half = divmod(c, 2)
        out[b, half * R : (half + 1) * R] = res.results[c]["out"]
    return out



# revision 5
# speedup vs baseline: 2.9228x; 2.9228x over previous
"""Linear attention Bass kernel for Trainium2 (8 NeuronCores).

Problem: x [4, 8192, 1024] f32, W [1024, 3072] f32.
  qkv = x @ W; q,k,v = split(qkv); q,k = elu(.)+1
  KV = einsum('bld,blh->bhd', k, v); ksum = k.sum(1)
  Z = 1/(q.ksum + eps); V = einsum('bld,bhd,bl->blh', q, KV, Z)

Sharding: 8 cores, core c handles batch b=c//2, sequence half h=c%2
(4096 rows each).  KV / ksum reductions span the full batch sequence, so
the two cores of a pair AllReduce their partial KV^T [1024,1024] + ksum
(4.2 MB fp32) in-NEFF.

Under axon the dispatch cost is dominated by host<->device transfer over
the tunnel (~100 MB/s), so the I/O layout is built to minimize bytes and
array count:
  - ONE input array per core: xpack [4096+384, 1024] bf16 = the core's x
    rows in natural layout plus a 1/8 column-shard of W (transposed
    [128,8,3072] layout, this core's 384-column slice).  W is re-assembled
    on device with an 8-way AllGather (6 MB over NeuronLink, negligible).
  - x is transposed ON DEVICE with XBAR transpose-DMAs (bf16), so the
    host does no strided transpose work.
  - Output is bf16 [4096, 1024] (halves both the donated zero-buffer
    upload and the result download); host upcasts to f32.

Per-core dataflow (all matmuls bf16 inputs, fp32 PSUM accumulation):
  phase 0: AllGather W shards -> wg; DMA into SBUF wsb [128,8,3072].
  phase 1: transpose-DMA xT tiles from xpack; q^T = Wq^T-form matmul
           (comes out [d,l] ready for phase 3), k,v = standard form
           [l,d]; phi=elu+1 via exp/min/max; q^T -> DRAM stash, k,v ->
           DRAM stash; ksum accumulated in PSUM via ones-vector matmul.
  phase 2: KV^T[d,h] += k_tile^T-free matmul over all l chunks, h in two
           512 halves (PSUM = 8 banks per half); partial KV^T + ksum ->
           cc buffer; AllReduce over core pairs.
  phase 3: V[l,:] = (q^T)^T @ KV^T, denominator from ksum column matmul,
           z = 1/(den+eps), scale, DMA out (bf16).
"""

import numpy as np
import ml_dtypes

import concourse.bass as bass
import concourse.tile as tile
from concourse import mybir
from concourse.bacc import Bacc

USE_CC = True
TRACE = False
LAST_RESULTS = None

B, L, D = 4, 8192, 1024
NCORES = 8
R = 4096              # rows per core
LT = 512              # l-tile width (columns of xT per tile)
WS = 384              # W columns per core shard (3072 / 8)
EPS = 1e-6

BF16 = mybir.dt.bfloat16
F32 = mybir.dt.float32
NPBF16 = ml_dtypes.bfloat16

_NC_CACHE = {}


def _emit_phi(nc, pool_e, out_bf, psum_in, width):
    """out_bf (bf16) = elu(psum_in)+1 = min(exp(y),1) + max(y,0).

    Ops are emitted per 512-wide slice so each reads a single PSUM bank
    (one stop-matmul dep); the combine reads only SBUF tiles.  Keeps the
    per-instruction semaphore-wait count under the ISA limit.
    """
    for s in range(0, width, 512):
        w = min(512, width - s)
        ps = psum_in[:, s : s + w]
        e = pool_e.tile([128, w], F32, tag=f"phi_e_{w}_{s}", name=f"e{w}_{s}")
        nc.scalar.activation(out=e, in_=ps, func=mybir.ActivationFunctionType.Exp)
        r = pool_e.tile([128, w], F32, tag=f"phi_r_{w}_{s}", name=f"r{w}_{s}")
        nc.vector.tensor_scalar(
            out=r, in0=ps, scalar1=0.0, scalar2=None, op0=mybir.AluOpType.max
        )
        nc.vector.scalar_tensor_tensor(
            out=out_bf[:, s : s + w],
            in0=e,
            scalar=1.0,
            in1=r,
            op0=mybir.AluOpType.min,
            op1=mybir.AluOpType.add,
        )


def build_bass(use_cc=True):
    nc = Bacc(trn_type="TRN2", num_devices=NCORES)

    n_lc = R // 128                  # 32 chunks of 128 rows
    n_tiles = R // LT                # 8 l-tiles

    # Single packed input: rows 0:4096 = x rows (natural layout), rows
    # 4096:4480 = this core's W shard, flat order p*3072 + k*384 + j.
    xpack = nc.dram_tensor("xpack", [R + WS, 1024], BF16, kind="ExternalInput")
    out = nc.dram_tensor("out", [R, 1024], BF16, kind="ExternalOutput")

    # AllGather target: wg[s] = shard s as [128 part, 8 kchunk, 384 cols].
    # Collectives may not read IO tensors, so the shard is staged through
    # an Internal DRAM tensor first (DRAM->DRAM DMA, 0.75 MB).
    wstage = nc.dram_tensor("wstage", [WS, 1024], BF16)
    wg = nc.dram_tensor("wg", [8, 128, 8, WS], BF16)

    q_dram = nc.dram_tensor("q_stash", [128, 8, R], BF16)
    k_dram = nc.dram_tensor("k_stash", [n_lc, 128, 1024], BF16)
    v_dram = nc.dram_tensor("v_stash", [n_lc, 128, 1024], BF16)
    # row 128 of each [129, 1024] chunk holds ksum[m*128:(m+1)*128] in
    # cols 0:128 (rest unread, harmlessly allreduced).
    cc_in = nc.dram_tensor("cc_in", [8, 129, 1024], F32)
    cc_out = nc.dram_tensor("cc_out", [8, 129, 1024], F32)

    mm = nc.tensor.matmul
    Act = mybir.ActivationFunctionType

    with tile.TileContext(nc) as tc:
        with tc.tile_pool(name="consts", bufs=1) as consts:
            # ---------------- phase 0: W AllGather + load ----------------
            nc.sync.dma_start(out=wstage[:], in_=xpack[R : R + WS, :])
            nc.gpsimd.collective_compute(
                "AllGather",
                mybir.AluOpType.bypass,
                replica_groups=[[0, 1, 2, 3, 4, 5, 6, 7]],
                ins=[wstage[:]],
                outs=[wg[:]],
            )
            wsb = consts.tile([128, 8, 3072], BF16)
            for s in range(8):
                nc.sync.dma_start(
                    out=wsb[:, :, s * WS : (s + 1) * WS], in_=wg[s]
                )
            ones_sb = consts.tile([128, 1], BF16)
            nc.vector.memset(ones_sb, 1.0)

            # ---------------- phase 1: qkv + phi + stashes + ksum ---------
            with (
                tc.tile_pool(name="xt_p", bufs=3) as xt_p,
                tc.tile_pool(name="qout_p", bufs=2) as qout_p,
                tc.tile_pool(name="e_p", bufs=4) as e_p,
                tc.tile_pool(name="kt_p", bufs=3) as kt_p,
                tc.tile_pool(name="vt_p", bufs=3) as vt_p,
                tc.tile_pool(name="q_ps_p", bufs=2, space="PSUM") as q_ps_p,
                tc.tile_pool(name="kv_ps_p", bufs=1, space="PSUM") as kv_ps_p,
                tc.tile_pool(name="ks_ps_p", bufs=1, space="PSUM") as ks_ps_p,
            ):
                ksum_ps = [
                    ks_ps_p.tile([1, 512], F32, tag=f"ks{h}", name=f"ks{h}")
                    for h in range(2)
                ]

                def q_block(xt_tile, qout, m):
                    pq = q_ps_p.tile([128, LT], F32)
                    for k in range(8):
                        mm(
                            pq,
                            lhsT=wsb[:, k, m * 128 : (m + 1) * 128],
                            rhs=xt_tile[:, k, :],
                            start=(k == 0),
                            stop=(k == 7),
                        )
                    _emit_phi(nc, e_p, qout[:, m, :], pq, LT)

                def kv_block(xt_tile, t, lc):
                    idx = t * 4 + lc
                    # four independent single-bank PSUM tiles: each reader
                    # then carries exactly one stop-matmul dependency.
                    pkv = [
                        kv_ps_p.tile([128, 512], F32, tag=f"pkv{n}", name=f"pkv{n}")
                        for n in range(4)
                    ]
                    for k in range(8):
                        lhsT = xt_tile[:, k, lc * 128 : (lc + 1) * 128]
                        for n in range(4):
                            mm(
                                pkv[n],
                                lhsT=lhsT,
                                rhs=wsb[:, k, 1024 + n * 512 : 1024 + (n + 1) * 512],
                                start=(k == 0),
                                stop=(k == 7),
                            )
                    kt = kt_p.tile([128, 1024], BF16)
                    for s in range(2):
                        _emit_phi(nc, e_p, kt[:, s * 512 : (s + 1) * 512], pkv[s], 512)
                    vt = vt_p.tile([128, 1024], BF16)
                    for s in range(2):
                        nc.scalar.activation(
                            out=vt[:, s * 512 : (s + 1) * 512],
                            in_=pkv[2 + s],
                            func=Act.Copy,
                        )
                    nc.sync.dma_start(out=k_dram[idx], in_=kt)
                    nc.sync.dma_start(out=v_dram[idx], in_=vt)
                    for h in range(2):
                        mm(
                            ksum_ps[h],
                            lhsT=ones_sb,
                            rhs=kt[:, h * 512 : (h + 1) * 512],
                            start=(idx == 0),
                            stop=(idx == n_lc - 1),
                        )

                for t in range(n_tiles):
                    xt_tile = xt_p.tile([128, 8, LT], BF16)
                    # xT tile via XBAR transpose-DMA straight from the
                    # natural-layout x rows: in [512 l, 128 d] -> out
                    # [128 d, 512 l].
                    for kd in range(8):
                        nc.sync.dma_start(
                            out=xt_tile[:, kd, :],
                            in_=xpack[t * LT : (t + 1) * LT, kd * 128 : (kd + 1) * 128],
                            transpose=True,
                        )
                    qout = qout_p.tile([128, 8, LT], BF16)
                    for seg in range(4):
                        q_block(xt_tile, qout, 2 * seg)
                        q_block(xt_tile, qout, 2 * seg + 1)
                        kv_block(xt_tile, t, seg)
                    nc.sync.dma_start(
                        out=q_dram[:, :, t * LT : (t + 1) * LT], in_=qout
                    )

                # stash ksum (psum) to DRAM before phase-1 psum pools close
                ks_sb = consts.tile([1, 1024], F32)
                for h in range(2):
                    nc.vector.tensor_copy(
                        out=ks_sb[:, h * 512 : (h + 1) * 512], in_=ksum_ps[h]
                    )
                zrow = consts.tile([1, 896], F32)
                nc.vector.memset(zrow, 0.0)
                for m in range(8):
                    nc.sync.dma_start(
                        out=cc_in[m, 128, 0:128],
                        in_=ks_sb[0:1, m * 128 : (m + 1) * 128],
                    )
                    nc.sync.dma_start(out=cc_in[m, 128, 128:1024], in_=zrow)

            # ---------------- phase 2: KV^T accumulation ------------------
            with (
                tc.tile_pool(name="k2_p", bufs=6) as k2_p,
                tc.tile_pool(name="v2_p", bufs=6) as v2_p,
                tc.tile_pool(name="kvt_ps_p", bufs=1, space="PSUM") as kvt_ps_p,
            ):
                for half in range(2):
                    kvt_ps = [
                        kvt_ps_p.tile(
                            [128, 512], F32, tag=f"kvt{m}", name=f"kvt{m}"
                        )
                        for m in range(8)
                    ]
                    for lc in range(n_lc):
                        kt2 = k2_p.tile([128, 1024], BF16)
                        nc.sync.dma_start(out=kt2, in_=k_dram[lc])
                        vt2 = v2_p.tile([128, 512], BF16)
                        nc.sync.dma_start(
                            out=vt2,
                            in_=v_dram[lc][:, half * 512 : (half + 1) * 512],
                        )
                        for m in range(8):
                            mm(
                                kvt_ps[m],
                                lhsT=kt2[:, m * 128 : (m + 1) * 128],
                                rhs=vt2,
                                start=(lc == 0),
                                stop=(lc == n_lc - 1),
                            )
                    for m in range(8):
                        kvs = k2_p.tile(
                            [128, 512], F32, tag="kvs", name=f"kvs{half}_{m}"
                        )
                        nc.scalar.activation(
                            out=kvs, in_=kvt_ps[m], func=Act.Copy
                        )
                        nc.sync.dma_start(
                            out=cc_in[m, 0:128, half * 512 : (half + 1) * 512],
                            in_=kvs,
                        )

            nc.gpsimd.collective_compute(
                "AllReduce",
                mybir.AluOpType.add,
                replica_groups=[[0, 1], [2, 3], [4, 5], [6, 7]],
                ins=[cc_in[:]],
                outs=[cc_out[:]],
            )

            # ---------------- phase 3: output -------------------------
            with (
                tc.tile_pool(name="p3", bufs=1) as p3,
                tc.tile_pool(name="qt_p", bufs=2) as qt_p,
                tc.tile_pool(name="ob_p", bufs=3) as ob_p,
                tc.tile_pool(name="z_p", bufs=4) as z_p,
                tc.tile_pool(name="pv_ps_p", bufs=2, space="PSUM") as pv_ps_p,
                tc.tile_pool(name="pd_ps_p", bufs=2, space="PSUM") as pd_ps_p,
            ):
                kvt_f = p3.tile([128, 8, 1024], F32)
                for m in range(8):
                    nc.sync.dma_start(
                        out=kvt_f[:, m, :], in_=cc_out[m, 0:128, :]
                    )
                kvt_bf = p3.tile([128, 8, 1024], BF16)
                for m in range(8):
                    nc.vector.tensor_copy(
                        out=kvt_bf[:, m, :], in_=kvt_f[:, m, :]
                    )
                ksum_f = p3.tile([128, 8], F32)
                for m in range(8):
                    nc.sync.dma_start(
                        out=ksum_f[:, m : m + 1], in_=cc_out[m, 128, 0:128]
                    )
                ksum_b = p3.tile([128, 8], BF16)
                for m in range(8):
                    nc.vector.tensor_copy(
                        out=ksum_b[:, m : m + 1], in_=ksum_f[:, m : m + 1]
                    )

                for g in range(8):
                    qt = qt_p.tile([128, 8, 512], BF16)
                    nc.sync.dma_start(
                        out=qt, in_=q_dram[:, :, g * 512 : (g + 1) * 512]
                    )
                    for lc in range(4):
                        pv0 = pv_ps_p.tile([128, 512], F32, tag="pv0")
                        pv1 = pv_ps_p.tile([128, 512], F32, tag="pv1")
                        pd = pd_ps_p.tile([128, 1], F32)
                        for k in range(8):
                            lhsT = qt[:, k, lc * 128 : (lc + 1) * 128]
                            st, sp = (k == 0), (k == 7)
                            mm(pv0, lhsT=lhsT, rhs=kvt_bf[:, k, 0:512],
                               start=st, stop=sp)
                            mm(pv1, lhsT=lhsT, rhs=kvt_bf[:, k, 512:1024],
                               start=st, stop=sp)
                            mm(pd, lhsT=lhsT, rhs=ksum_b[:, k : k + 1],
                               start=st, stop=sp)
                        z = z_p.tile([128, 1], F32)
                        nc.vector.tensor_scalar(
                            out=z, in0=pd, scalar1=EPS, scalar2=None,
                            op0=mybir.AluOpType.add,
                        )
                        nc.vector.reciprocal(out=z, in_=z)
                        ob = ob_p.tile([128, 1024], BF16)
                        nc.vector.tensor_scalar_mul(
                            out=ob[:, 0:512], in0=pv0, scalar1=z
                        )
                        nc.vector.tensor_scalar_mul(
                            out=ob[:, 512:1024], in0=pv1, scalar1=z
                        )
                        r0 = (g * 4 + lc) * 128
                        nc.sync.dma_start(out=out[r0 : r0 + 128, :], in_=ob)
    if not nc.is_finalized():
        nc.finalize()
    return nc


def _get_nc(use_cc=True):
    key = True  # single variant
    if key not in _NC_CACHE:
        _NC_CACHE[key] = build_bass(key)
    return _NC_CACHE[key]


def _prep_inputs(x, W, use_cc=True):
    """Build per-core packed inputs (cheap: casts + contiguous copies only)."""
    xbf = np.asarray(x, np.float32).reshape(NCORES, R, D).astype(NPBF16)
    # W -> [128 part, 8 kchunk, 3072 col] layout, then per-core 384-col shard
    wt = np.ascontiguousarray(
        np.asarray(W, np.float32).reshape(8, 128, 3 * D).transpose(1, 0, 2)
    ).astype(NPBF16)
    in_maps = []
    for c in range(NCORES):
        xp = np.empty((R + WS, D), NPBF16)
        xp[:R] = xbf[c]
        xp[R:] = np.ascontiguousarray(
            wt[:, :, c * WS : (c + 1) * WS]
        ).reshape(WS, D)
        in_maps.append({"xpack": xp})
    return in_maps


def kernel(x, W):
    global LAST_RESULTS
    from concourse.bass_utils import run_bass_kernel_spmd

    nc = _get_nc(True)
    in_maps = _prep_inputs(x, W)
    try:
        res = run_bass_kernel_spmd(
            nc, in_maps, core_ids=list(range(NCORES)), trace=TRACE
        )
    except ModuleNotFoundError:
        # NTFF profiling hook unavailable (axon client without antenv.axon_hooks)
        res = run_bass_kernel_spmd(
            nc, in_maps, core_ids=list(range(NCORES)), trace=False
        )
    LAST_RESULTS = res
    out = np.empty((B, L, D), dtype=np.float32)
    for c in range(NCORES):
        b, half = divmod(c, 2)
        out[b, half * R : (half + 1) * R] = res.results[c]["out"].astype(np.float32)
    return out
